# revision 25
# baseline (speedup 1.0000x reference)
"""Trainium2 Bass kernel for nn_Averager (pooling, 3-level box-average).

Math (verified vs reference): per sample, with input x[n, i, c] where
n = (n5 n4 n3 n2 n1 n0) base-4 digits, c = (c2 c1 c0) base-4 digits:
  out[:, :, 0, :] = x[:, :, 0, :]
  out1[n, c] = E[n4, n2, c2, c0, n0, c1],
      E[r5, r4, r3, r0; g2, g1] = mean over (n2, n1, c0) of x1
  out2[n, c] = G[c2, c1, c0],
      G[p, q, r] = mean over (n4, n3, n1, n0, c1, c0) of x2 with
      (n5, c2in, n2) = (p, q, r)

Sharding: data-parallel over batch, 4 samples per core on 8 cores,
processed as 2 groups of 2 samples.

Layout (pair-contiguous): SBUF partition p = b*64 + n//64 =
(b, n5, n4, n3); free j = n % 64 = 16*n2 + 4*n1 + n0, row (i, c).
A 6MB group is contiguous in DRAM and per-partition contiguous in
SBUF: each group is ONE 2-D in-DMA split in 2 j-halves and 2-3
contiguous out-DMAs.  DMA descriptors are >=12KB per partition: the
queue engines spend ~0.5-0.7us per descriptor regardless of size, so
24KB descriptors are needed to saturate HBM (~430GB/s agg); per-level
region DMAs (256B descriptors) were the original bottleneck, and even
a 4-way quarter split (12KB) caps at ~320GB/s.

All in-DMAs ride SWDGE on GpSimd and are triggered upfront (g0h0,
g0h1, s12, g1h0, g1h1) so the queues drain them back-to-back.  All
out-DMAs ride the single Sync HWDGE ring (FIFO), headed by a 4-byte
dummy "gate" DMA that reads the tail of the last input tile: the out
descriptors start exactly when the input stream finishes, never
earlier.  Queue arbitration round-robins rings per-descriptor, so an
early out trigger would otherwise steal queue bandwidth from the
input tail and stall the next group's compute (measured +17us).  The
gate also phase-separates reads from writes device-wide (all 8 cores
run the same schedule), which keeps HBM in clean streaming mode.

Engine split per group:
  DVE : L1 lane-local sums (u0/u1/w/h1/h2 -> A, bf16) and L2 sums
        (r01/r23 -> A2, bf16; split by j-half so the r01 reduce fills
        the stall between the two in-DMA halves).
  PE  : 16 L1 routing matmuls + 4 L2 reduce matmuls, 1-pass (lhsT =
        fp8e5 selectors — 1/64 and 1/4096 are exact powers of two —
        with bf16 rhs; fp32 would take 2 LDWEIGHTS+MATMUL passes).
        The L1 matmuls write PSUM through a strided out-AP so PSUM
        free = (n2, n0, c) matches the output interleave.
  ACT : PSUM evacuations (256B-contiguous runs, n1 via step-0 src dim)
        and the on-chip broadcast of the single L2 row G into all 64
        j-rows — runs parallel to DVE working on the next group.
Phase-1 (stage-A + matmuls, both groups) is emitted before phase-2
(evac + flush) so the framework's stage-reset barriers — which wait
on all earlier DMAs — never land inside the critical DVE chain.
Outputs are assembled IN-PLACE into the input tile (L0 rows pass
through untouched) and flushed chunk-by-chunk as each chunk's
evac+broadcast completes; the last group tapers to quarter-chunks so
the final post-compute drain is half as long.
"""

import numpy as np

N_CORES = 8
B_FULL = 32
B_CORE = B_FULL // N_CORES  # 4
N = 4096
LVL = 3
C = 64


def _make_selectors():
    """Routing selectors, pair layout: k = 64*b + 16*k5 + 4*k4 + k3.

    S1 block (n2o, c2o), 16 blocks:
        S1[k, m] = 1/64   iff b(k)==b(m), k5==m4, k4==n2o, k3==c2o
    S2 block (c2o), 4 blocks:
        S2[k, m] = 1/4096 iff b(k)==b(m), k5==c2o
    """
    k = np.arange(128)
    b, k5, k4, k3 = k >> 6, (k >> 4) & 3, (k >> 2) & 3, k & 3
    m = np.arange(128)
    bm, m4 = m >> 6, (m >> 2) & 3
    S1 = np.zeros((128, 16, 128), np.float32)
    S2 = np.zeros((128, 4, 128), np.float32)
    for n2o in range(4):
        for c2o in range(4):
            S1[:, n2o * 4 + c2o, :] = (
                (b[:, None] == bm[None, :])
                & (k5[:, None] == m4[None, :])
                & (k4[:, None] == n2o)
                & (k3[:, None] == c2o)
            ).astype(np.float32) / 64.0
    for c2o in range(4):
        S2[:, c2o, :] = (
            (b[:, None] == bm[None, :]) & (k5[:, None] == c2o)
        ).astype(np.float32) / 4096.0
    return (
        np.ascontiguousarray(S1.reshape(128, 2048)),
        np.ascontiguousarray(S2.reshape(128, 512)),
    )


def _build_nc():
    import concourse.bass as bass
    import concourse.tile as tile
    from concourse import mybir

    dt = mybir.dt.float32
    bf = mybir.dt.bfloat16
    f8 = mybir.dt.float8e5
    X = mybir.AxisListType.X
    ADD = mybir.AluOpType.add

    from concourse import bacc
    nc = bacc.Bacc()
    x = nc.declare_dram_parameter("x", [B_CORE, N, LVL, C], dt, isOutput=False)
    s12 = nc.declare_dram_parameter("s12", [128, 2560], f8, isOutput=False)
    out = nc.declare_dram_parameter("out", [B_CORE, N, LVL, C], dt, isOutput=True)

    NG = B_CORE // 2

    with tile.TileContext(nc) as tc:
        with (
            tc.tile_pool(name="consts", bufs=1) as cpool,
            tc.tile_pool(name="xin", bufs=2) as xpool,
            tc.tile_pool(name="tmp", bufs=1) as tpool,
            tc.tile_pool(name="psum", bufs=2, space="PSUM") as ppool,
        ):
            # ---- all input DMAs upfront: x group 0 halves, selectors,
            # x group 1 halves, drained back-to-back in this order ----
            xts_ = []
            for g in range(NG):
                xt = xpool.tile([128, 12288], dt, tag="xt")
                xsrc = x[2 * g:2 * g + 2].rearrange(
                    "b (ph j) i c -> (b ph) (j i c)", ph=64
                )
                nc.gpsimd.dma_start(xt[:, 0:6144], xsrc[:, 0:6144])
                nc.gpsimd.dma_start(xt[:, 6144:12288], xsrc[:, 6144:12288])
                xts_.append(xt)
                if g == 0:
                    s12sb = cpool.tile([128, 2560], f8, tag="s12")
                    nc.gpsimd.dma_start(s12sb[:], s12[:])
            s1sb = s12sb[:, 0:2048]
            s2sb = s12sb[:, 2048:2560]

            # ---- phase 1 (both groups): DVE stage-A + PE matmuls.
            # Emitted before any out-DMA so the framework's stage-reset
            # barriers (which wait on all earlier DMAs) never insert an
            # out-DMA wait into the critical DVE chain. ----
            c1ps, gps = [], []
            for g in range(NG):
                xt = xts_[g]
                v = xt[:].rearrange(
                    "p (n2 n1 n0 i c) -> p n2 n1 n0 i c",
                    n2=4, n1=4, n0=4, i=3, c=64,
                )
                xw = xt[:].rearrange(
                    "p (j i c2 cc) -> p j i c2 cc", j=64, i=3, c2=4, cc=16
                )

                # ---- stage A per j-half (each needs only that in-half):
                # L1: u = fold n2-pairs of i=1 rows
                # L2: r = fold (c1 c0) of i=2 rows ----
                u0 = tpool.tile([128, 1024], dt, tag="u0")
                nc.vector.tensor_add(
                    u0[:].rearrange("p (n1 n0 c) -> p n1 n0 c", n1=4, n0=4, c=64),
                    v[:, 0, :, :, 1, :], v[:, 1, :, :, 1, :],
                )
                r01 = tpool.tile([128, 128], dt, tag="r01")
                nc.vector.tensor_reduce(
                    r01[:].rearrange("p (j c2) -> p j c2", j=32, c2=4),
                    xw[:, 0:32, 2, :, :],
                    axis=X, op=ADD,
                )
                u1 = tpool.tile([128, 1024], dt, tag="u1")
                nc.vector.tensor_add(
                    u1[:].rearrange("p (n1 n0 c) -> p n1 n0 c", n1=4, n0=4, c=64),
                    v[:, 2, :, :, 1, :], v[:, 3, :, :, 1, :],
                )
                r23 = tpool.tile([128, 128], dt, tag="r23")
                nc.vector.tensor_reduce(
                    r23[:].rearrange("p (j c2) -> p j c2", j=32, c2=4),
                    xw[:, 32:64, 2, :, :],
                    axis=X, op=ADD,
                )

                # ---- cross-half folds -> A (L1) and A2 (L2), bf16 ----
                w = tpool.tile([128, 1024], dt, tag="w")
                nc.vector.tensor_add(w[:], u0[:], u1[:])
                h1 = tpool.tile([128, 512], dt, tag="h1")
                nc.vector.tensor_add(h1[:], w[:, 0:512], w[:, 512:1024])
                h2 = tpool.tile([128, 256], dt, tag="h2")
                nc.vector.tensor_add(h2[:], h1[:, 0:256], h1[:, 256:512])
                # reduce c0, write A with free = 16*c2 + 4*c1 + n0 (bf16;
                # only 4-16 values accumulate so bf16 rounding ~0.4% << tol)
                A = tpool.tile([128, 64], bf, tag="A")
                A2 = tpool.tile([128, 16], bf, tag="A2")
                A2r = A2[:].rearrange("p (c2 n2) -> p n2 c2", c2=4, n2=4)
                with nc.allow_low_precision(reason="bf16 matmul rhs, tol 2e-2"):
                    nc.vector.tensor_reduce(
                        A[:].rearrange("p (c2 c1 n0) -> p n0 c2 c1", c2=4, c1=4, n0=4),
                        h2[:].rearrange(
                            "p (n0 c2 c1 c0) -> p n0 c2 c1 c0", n0=4, c2=4, c1=4, c0=4
                        ),
                        axis=X, op=ADD,
                    )
                    for n2 in range(4):
                        rr = r01 if n2 < 2 else r23
                        nc.vector.tensor_reduce(
                            A2r[:, n2, :],
                            rr[:, 64 * (n2 % 2):64 * (n2 % 2) + 64].rearrange(
                                "p (nn c2) -> p c2 nn", nn=16, c2=4
                            ),
                            axis=X, op=ADD,
                        )

                # ---- L1: 16 routing matmuls -> c1p psum ----
                # strided out-AP so psum free = 256*n2 + 64*n0 + 16*c2 +
                # (4*c1 + c0): the value for output digits (n2,n0,c2,c1,c0)
                c1p = ppool.tile([128, 1024], dt, tag="c1p")
                c1pv = c1p[:].rearrange(
                    "p (n2 n0 c2 cc) -> p n2 n0 c2 cc", n2=4, n0=4, c2=4, cc=16
                )
                for n2o in range(4):
                    for c2o in range(4):
                        blk = n2o * 4 + c2o
                        nc.tensor.matmul(
                            c1pv[:, n2o, :, c2o, :],
                            s1sb[:, blk * 128:(blk + 1) * 128],
                            A[:, 0:64],
                            start=True, stop=True,
                        )
                # ---- L2: 4 reduce+broadcast matmuls -> gp psum ----
                # gp free = 16*c2o + (4*c1o + c0o); rhs j = (c2in, n2)
                gp = ppool.tile([128, 64], dt, tag="gp")
                for c2o in range(4):
                    nc.tensor.matmul(
                        gp[:, c2o * 16:(c2o + 1) * 16],
                        s2sb[:, c2o * 128:(c2o + 1) * 128],
                        A2[:, 0:16],
                        start=True, stop=True,
                    )
                c1ps.append(c1p)
                gps.append(gp)

            # ---- phase 2 (both groups): ACT evac + flush.  All out-DMAs
            # ride the single Sync HWDGE ring (FIFO), headed by a 4-byte
            # dummy that reads the tail of the LAST input tile: out
            # descriptors therefore start exactly when the input stream
            # finishes, never earlier — an early out trigger would steal
            # queue bandwidth from the input tail (round-robin
            # arbitration) and stall the next group's compute. ----
            gate = tpool.tile([1, 1], dt, tag="gate")
            nc.sync.dma_start(gate[0:1, 0:1], xts_[NG - 1][127:128, 12287:12288])
            for g in range(NG):
                xt = xts_[g]
                c1p = c1ps[g]
                gp = gps[g]
                xtv = xt[:].rearrange(
                    "p (j i c) -> p j i c", j=64, i=3, c=64
                )
                # ---- ACT: PSUM evac + L2 row broadcast + flush, chunked
                # so each out-DMA launches while the next chunk evacuates.
                # Runs parallel to DVE working on the next group.  The
                # last group tapers to quarter-chunks to shrink the final
                # post-compute drain. ----
                c1e = c1p[:].rearrange(
                    "p (n2 o n0 c) -> p n2 o n0 c", n2=4, o=1, n0=4, c=64
                )
                xto = xt[:].rearrange(
                    "p (n2 n1 n0 i c) -> p n2 n1 n0 i c",
                    n2=4, n1=4, n0=4, i=3, c=64,
                )
                gpb = gp[:].rearrange("p (o c) -> p o c", o=1)
                outv = out[2 * g:2 * g + 2].rearrange(
                    "b (ph j) i c -> (b ph) (j i c)", ph=64
                )
                # chunks of n2 quarters; group 0 flushes as ONE DMA with
                # 48KB-per-partition descriptors (it is fully evacuated
                # before the gate opens, and bigger descriptors amortize
                # the ~0.24us fixed per-descriptor queue cost); the last
                # group tapers to quarter chunks so the final post-compute
                # drain is half as long
                if g < NG - 1:
                    chunks = [(0, 1, 2, 3)]
                else:
                    chunks = [(0, 1), (2,), (3,)]
                for n2s in chunks:
                    for n2o in n2s:
                        nc.scalar.copy(
                            xto[:, n2o, :, :, 1, :],
                            c1e[:, n2o, :, :, :].broadcast_to((128, 4, 4, 64)),
                        )
                    jlo, jhi = 16 * n2s[0], 16 * n2s[-1] + 16
                    nc.scalar.copy(
                        xtv[:, jlo:jhi, 2, :],
                        gpb.broadcast_to((128, jhi - jlo, 64)),
                    )
                    nc.sync.dma_start(
                        outv[:, 192 * jlo:192 * jhi],
                        xt[:, 192 * jlo:192 * jhi],
                    )
    nc.compile()
    return nc


_NC_CACHE = {}


def _get_nc():
    if "nc" not in _NC_CACHE:
        _NC_CACHE["nc"] = _build_nc()
    return _NC_CACHE["nc"]


def kernel(**inputs: np.ndarray) -> np.ndarray:
    import ml_dtypes
    from concourse.bass_utils import run_bass_kernel_spmd

    x = np.ascontiguousarray(inputs["x"], dtype=np.float32)
    assert x.shape == (B_FULL, N, LVL, C), x.shape
    S1, S2 = _make_selectors()
    S12 = np.ascontiguousarray(
        np.concatenate([S1, S2], axis=1).astype(ml_dtypes.float8_e5m2)
    )
    nc = _get_nc()
    in_maps = [
        {"x": np.ascontiguousarray(x[k * B_CORE:(k + 1) * B_CORE]),
         "s12": S12}
        for k in range(N_CORES)
    ]
    res = run_bass_kernel_spmd(nc, in_maps, list(range(N_CORES)))
    outs = [res.results[k]["out"] for k in range(N_CORES)]
    return np.ascontiguousarray(np.concatenate(outs, axis=0))


# revision 26
# speedup vs baseline: 1.2115x; 1.2115x over previous
"""Trainium2 Bass kernel for nn_Averager (pooling, 3-level box-average).

Math (verified vs reference): per sample, with input x[n, i, c] where
n = (n5 n4 n3 n2 n1 n0) base-4 digits, c = (c2 c1 c0) base-4 digits:
  out[:, :, 0, :] = x[:, :, 0, :]
  out1[n, c] = E[n4, n2, c2, c0, n0, c1],
      E[r5, r4, r3, r0; g2, g1] = mean over (n2, n1, c0) of x1
  out2[n, c] = G[c2, c1, c0],
      G[p, q, r] = mean over (n4, n3, n1, n0, c1, c0) of x2 with
      (n5, c2in, n2) = (p, q, r)

Sharding: data-parallel over batch, 4 samples per core on 8 cores,
processed as 2 groups of 2 samples.

Layout (pair-contiguous): SBUF partition p = b*64 + n//64 =
(b, n5, n4, n3); free j = n % 64 = 16*n2 + 4*n1 + n0, row (i, c).
A 6MB group is contiguous in DRAM and per-partition contiguous in
SBUF: each group is ONE 2-D in-DMA split in 2 j-halves and 2-3
contiguous out-DMAs.  DMA descriptors are >=12KB per partition: the
queue engines spend ~0.5-0.7us per descriptor regardless of size, so
24KB descriptors are needed to saturate HBM (~430GB/s agg); per-level
region DMAs (256B descriptors) were the original bottleneck, and even
a 4-way quarter split (12KB) caps at ~320GB/s.

All in-DMAs ride SWDGE on GpSimd and are triggered upfront (g0h0,
g0h1, s12, g1h0, g1h1) so the queues drain them back-to-back.  All
out-DMAs ride the single Sync HWDGE ring (FIFO), headed by a 4-byte
dummy "gate" DMA that reads the tail of the last input tile: the out
descriptors start exactly when the input stream finishes, never
earlier.  Queue arbitration round-robins rings per-descriptor, so an
early out trigger would otherwise steal queue bandwidth from the
input tail and stall the next group's compute (measured +17us).  The
gate also phase-separates reads from writes device-wide (all 8 cores
run the same schedule), which keeps HBM in clean streaming mode.

Engine split per group:
  DVE : L1 lane-local sums (u0/u1/w/h1/h2 -> A, bf16) and L2 sums
        (r01/r23 -> A2, bf16; split by j-half so the r01 reduce fills
        the stall between the two in-DMA halves).
  PE  : 16 L1 routing matmuls + 4 L2 reduce matmuls, 1-pass (lhsT =
        fp8e5 selectors — 1/64 and 1/4096 are exact powers of two —
        with bf16 rhs; fp32 would take 2 LDWEIGHTS+MATMUL passes).
        The L1 matmuls write PSUM through a strided out-AP so PSUM
        free = (n2, n0, c) matches the output interleave.
  ACT : PSUM evacuations (256B-contiguous runs, n1 via step-0 src dim)
        and the on-chip broadcast of the single L2 row G into all 64
        j-rows — runs parallel to DVE working on the next group.
Phase-1 (stage-A + matmuls, both groups) is emitted before phase-2
(evac + flush) so the framework's stage-reset barriers — which wait
on all earlier DMAs — never land inside the critical DVE chain.
Outputs are assembled IN-PLACE into the input tile (L0 rows pass
through untouched) and flushed chunk-by-chunk as each chunk's
evac+broadcast completes; the last group tapers to quarter-chunks so
the final post-compute drain is half as long.
"""

import numpy as np

N_CORES = 8
B_FULL = 32
B_CORE = B_FULL // N_CORES  # 4
N = 4096
LVL = 3
C = 64


def _make_selectors():
    """Routing selectors, pair layout: k = 64*b + 16*k5 + 4*k4 + k3.

    S1 block (n2o, c2o), 16 blocks:
        S1[k, m] = 1/64   iff b(k)==b(m), k5==m4, k4==n2o, k3==c2o
    S2 block (c2o), 4 blocks:
        S2[k, m] = 1/4096 iff b(k)==b(m), k5==c2o
    """
    k = np.arange(128)
    b, k5, k4, k3 = k >> 6, (k >> 4) & 3, (k >> 2) & 3, k & 3
    m = np.arange(128)
    bm, m4 = m >> 6, (m >> 2) & 3
    S1 = np.zeros((128, 16, 128), np.float32)
    S2 = np.zeros((128, 4, 128), np.float32)
    for n2o in range(4):
        for c2o in range(4):
            S1[:, n2o * 4 + c2o, :] = (
                (b[:, None] == bm[None, :])
                & (k5[:, None] == m4[None, :])
                & (k4[:, None] == n2o)
                & (k3[:, None] == c2o)
            ).astype(np.float32) / 64.0
    for c2o in range(4):
        S2[:, c2o, :] = (
            (b[:, None] == bm[None, :]) & (k5[:, None] == c2o)
        ).astype(np.float32) / 4096.0
    return (
        np.ascontiguousarray(S1.reshape(128, 2048)),
        np.ascontiguousarray(S2.reshape(128, 512)),
    )


def _build_nc():
    import concourse.bass as bass
    import concourse.tile as tile
    from concourse import mybir

    dt = mybir.dt.float32
    bf = mybir.dt.bfloat16
    f8 = mybir.dt.float8e5
    X = mybir.AxisListType.X
    ADD = mybir.AluOpType.add

    from concourse import bacc
    nc = bacc.Bacc()
    x = nc.declare_dram_parameter("x", [B_CORE, N, LVL, C], dt, isOutput=False)
    s12 = nc.declare_dram_parameter("s12", [128, 2560], f8, isOutput=False)
    out = nc.declare_dram_parameter("out", [B_CORE, N, LVL, C], dt, isOutput=True)

    NG = B_CORE // 2

    with tile.TileContext(nc) as tc:
        with (
            tc.tile_pool(name="consts", bufs=1) as cpool,
            tc.tile_pool(name="xin", bufs=2) as xpool,
            tc.tile_pool(name="tmp", bufs=1) as tpool,
            tc.tile_pool(name="psum", bufs=2, space="PSUM") as ppool,
        ):
            # ---- all input DMAs upfront: x group 0 halves, selectors,
            # x group 1 halves, drained back-to-back in this order ----
            xts_ = []
            for g in range(NG):
                xt = xpool.tile([128, 12288], dt, tag="xt")
                xsrc = x[2 * g:2 * g + 2].rearrange(
                    "b (ph j) i c -> (b ph) (j i c)", ph=64
                )
                nc.gpsimd.dma_start(xt[:, 0:6144], xsrc[:, 0:6144])
                nc.gpsimd.dma_start(xt[:, 6144:12288], xsrc[:, 6144:12288])
                xts_.append(xt)
                if g == 0:
                    s12sb = cpool.tile([128, 2560], f8, tag="s12")
                    nc.gpsimd.dma_start(s12sb[:], s12[:])
            s1sb = s12sb[:, 0:2048]
            s2sb = s12sb[:, 2048:2560]

            # ---- phase 1 (both groups): DVE stage-A + PE matmuls.
            # Emitted before any out-DMA so the framework's stage-reset
            # barriers (which wait on all earlier DMAs) never insert an
            # out-DMA wait into the critical DVE chain. ----
            c1ps, gps = [], []
            for g in range(NG):
                xt = xts_[g]
                v = xt[:].rearrange(
                    "p (n2 n1 n0 i c) -> p n2 n1 n0 i c",
                    n2=4, n1=4, n0=4, i=3, c=64,
                )
                xw = xt[:].rearrange(
                    "p (j i c2 cc) -> p j i c2 cc", j=64, i=3, c2=4, cc=16
                )

                # ---- stage A per j-half (each needs only that in-half):
                # L1: u = fold n2-pairs of i=1 rows
                # L2: r = fold (c1 c0) of i=2 rows ----
                u0 = tpool.tile([128, 1024], dt, tag="u0")
                nc.vector.tensor_add(
                    u0[:].rearrange("p (n1 n0 c) -> p n1 n0 c", n1=4, n0=4, c=64),
                    v[:, 0, :, :, 1, :], v[:, 1, :, :, 1, :],
                )
                r01 = tpool.tile([128, 128], dt, tag="r01")
                nc.vector.tensor_reduce(
                    r01[:].rearrange("p (j c2) -> p j c2", j=32, c2=4),
                    xw[:, 0:32, 2, :, :],
                    axis=X, op=ADD,
                )
                u1 = tpool.tile([128, 1024], dt, tag="u1")
                nc.vector.tensor_add(
                    u1[:].rearrange("p (n1 n0 c) -> p n1 n0 c", n1=4, n0=4, c=64),
                    v[:, 2, :, :, 1, :], v[:, 3, :, :, 1, :],
                )
                r23 = tpool.tile([128, 128], dt, tag="r23")
                nc.vector.tensor_reduce(
                    r23[:].rearrange("p (j c2) -> p j c2", j=32, c2=4),
                    xw[:, 32:64, 2, :, :],
                    axis=X, op=ADD,
                )

                # ---- cross-half folds -> A (L1) and A2 (L2), bf16 ----
                w = tpool.tile([128, 1024], dt, tag="w")
                nc.vector.tensor_add(w[:], u0[:], u1[:])
                h1 = tpool.tile([128, 512], dt, tag="h1")
                nc.vector.tensor_add(h1[:], w[:, 0:512], w[:, 512:1024])
                h2 = tpool.tile([128, 256], dt, tag="h2")
                nc.vector.tensor_add(h2[:], h1[:, 0:256], h1[:, 256:512])
                # reduce c0, write A with free = 16*c2 + 4*c1 + n0 (bf16;
                # only 4-16 values accumulate so bf16 rounding ~0.4% << tol)
                A = tpool.tile([128, 64], bf, tag="A")
                A2 = tpool.tile([128, 16], bf, tag="A2")
                A2r = A2[:].rearrange("p (c2 n2) -> p n2 c2", c2=4, n2=4)
                with nc.allow_low_precision(reason="bf16 matmul rhs, tol 2e-2"):
                    nc.vector.tensor_reduce(
                        A[:].rearrange("p (c2 c1 n0) -> p n0 c2 c1", c2=4, c1=4, n0=4),
                        h2[:].rearrange(
                            "p (n0 c2 c1 c0) -> p n0 c2 c1 c0", n0=4, c2=4, c1=4, c0=4
                        ),
                        axis=X, op=ADD,
                    )
                    for n2 in range(4):
                        rr = r01 if n2 < 2 else r23
                        nc.vector.tensor_reduce(
                            A2r[:, n2, :],
                            rr[:, 64 * (n2 % 2):64 * (n2 % 2) + 64].rearrange(
                                "p (nn c2) -> p c2 nn", nn=16, c2=4
                            ),
                            axis=X, op=ADD,
                        )

                # ---- L1: 16 routing matmuls -> c1p psum ----
                # strided out-AP so psum free = 256*n2 + 64*n0 + 16*c2 +
                # (4*c1 + c0): the value for output digits (n2,n0,c2,c1,c0)
                c1p = ppool.tile([128, 1024], dt, tag="c1p")
                c1pv = c1p[:].rearrange(
                    "p (n2 n0 c2 cc) -> p n2 n0 c2 cc", n2=4, n0=4, c2=4, cc=16
                )
                for n2o in range(4):
                    for c2o in range(4):
                        blk = n2o * 4 + c2o
                        nc.tensor.matmul(
                            c1pv[:, n2o, :, c2o, :],
                            s1sb[:, blk * 128:(blk + 1) * 128],
                            A[:, 0:64],
                            start=True, stop=True,
                        )
                # ---- L2: 4 reduce+broadcast matmuls -> gp psum ----
                # gp free = 16*c2o + (4*c1o + c0o); rhs j = (c2in, n2)
                gp = ppool.tile([128, 64], dt, tag="gp")
                for c2o in range(4):
                    nc.tensor.matmul(
                        gp[:, c2o * 16:(c2o + 1) * 16],
                        s2sb[:, c2o * 128:(c2o + 1) * 128],
                        A2[:, 0:16],
                        start=True, stop=True,
                    )
                c1ps.append(c1p)
                gps.append(gp)

            # ---- phase 2 (both groups): ACT evac + flush.  All out-DMAs
            # ride the single Sync HWDGE ring (FIFO), headed by a 4-byte
            # dummy that reads the tail of the LAST input tile: out
            # descriptors therefore start exactly when the input stream
            # finishes, never earlier — an early out trigger would steal
            # queue bandwidth from the input tail (round-robin
            # arbitration) and stall the next group's compute. ----
            gate = tpool.tile([1, 1], dt, tag="gate")
            nc.sync.dma_start(gate[0:1, 0:1], xts_[NG - 1][127:128, 12287:12288])
            for g in range(NG):
                xt = xts_[g]
                c1p = c1ps[g]
                gp = gps[g]
                xtv = xt[:].rearrange(
                    "p (j i c) -> p j i c", j=64, i=3, c=64
                )
                # ---- ACT: PSUM evac + L2 row broadcast + flush, chunked
                # so each out-DMA launches while the next chunk evacuates.
                # Runs parallel to DVE working on the next group.  The
                # last group tapers to quarter-chunks to shrink the final
                # post-compute drain. ----
                c1e = c1p[:].rearrange(
                    "p (n2 o n0 c) -> p n2 o n0 c", n2=4, o=1, n0=4, c=64
                )
                xto = xt[:].rearrange(
                    "p (n2 n1 n0 i c) -> p n2 n1 n0 i c",
                    n2=4, n1=4, n0=4, i=3, c=64,
                )
                gpb = gp[:].rearrange("p (o c) -> p o c", o=1)
                outv = out[2 * g:2 * g + 2].rearrange(
                    "b (ph j) i c -> (b ph) (j i c)", ph=64
                )
                # chunks of n2 quarters; the last group tapers to quarter
                # chunks so the final post-compute drain is half as long.
                # (Descriptor size >=12KB is bandwidth-neutral: measured
                # 460/908/1814ns for 12/24/48KB = flat 27GB/s per queue.)
                if g < NG - 1:
                    chunks = [(0, 1), (2, 3)]
                else:
                    chunks = [(0, 1), (2,), (3,)]
                for n2s in chunks:
                    for n2o in n2s:
                        nc.scalar.copy(
                            xto[:, n2o, :, :, 1, :],
                            c1e[:, n2o, :, :, :].broadcast_to((128, 4, 4, 64)),
                        )
                    jlo, jhi = 16 * n2s[0], 16 * n2s[-1] + 16
                    nc.scalar.copy(
                        xtv[:, jlo:jhi, 2, :],
                        gpb.broadcast_to((128, jhi - jlo, 64)),
                    )
                    nc.sync.dma_start(
                        outv[:, 192 * jlo:192 * jhi],
                        xt[:, 192 * jlo:192 * jhi],
                    )
    nc.compile()
    return nc


_NC_CACHE = {}


def _get_nc():
    if "nc" not in _NC_CACHE:
        _NC_CACHE["nc"] = _build_nc()
    return _NC_CACHE["nc"]


def kernel(**inputs: np.ndarray) -> np.ndarray:
    import ml_dtypes
    from concourse.bass_utils import run_bass_kernel_spmd

    x = np.ascontiguousarray(inputs["x"], dtype=np.float32)
    assert x.shape == (B_FULL, N, LVL, C), x.shape
    S1, S2 = _make_selectors()
    S12 = np.ascontiguousarray(
        np.concatenate([S1, S2], axis=1).astype(ml_dtypes.float8_e5m2)
    )
    nc = _get_nc()
    in_maps = [
        {"x": np.ascontiguousarray(x[k * B_CORE:(k + 1) * B_CORE]),
         "s12": S12}
        for k in range(N_CORES)
    ]
    res = run_bass_kernel_spmd(nc, in_maps, list(range(N_CORES)))
    outs = [res.results[k]["out"] for k in range(N_CORES)]
    return np.ascontiguousarray(np.concatenate(outs, axis=0))
